# revision 25
# baseline (speedup 1.0000x reference)
"""BitNet decoder MLP on 8 Trainium2 NeuronCores (Bass/Tile).

Strategy: data-parallel over batch (512 rows/core). Weights are ternary-
quantized cooperatively: each core owns the ic-slice [r*q, (r+1)*q) of every
output block of every layer. Two AllReduces supply the |W| means (L0's needs
only 2MB/core of reads, so it rides right behind the kernel-entry barrier and
unblocks L0's quant+AllGather while the L1-3 abs stream is still in flight);
each core quantizes its slice to an fp8e4 {-1,0,1} image, and per-layer
ob-group-chunked AllGathers (1/4/2/2 chunks) ship complete output-column
groups in consumption order so layer-l matmuls start as soon as their first
ob-group lands.

All matmul arithmetic is exact: activations are int8-valued bf16 (stationary),
weights are {-1,0,1} fp8e4 (moving) -- the mixed-dtype matmul is bit-exact on
TRN2 hardware and halves both AllGather bytes and main-pass weight DMA vs
bf16. Accumulation is fp32 in PSUM. Per-row dequant scales fold into the
PSUM-eviction pass on the scalar engine; LayerNorm+SiLU run as fused
scalar-engine passes (sigmoid via the tanh table); rsqrt via Newton on the
vector engine; rounding via the fp32 magic-number trick (round-half-even).

The main pass runs each layer in two batch-tile phases (L1: {0,1,2}/{3} so
its trailing AllGather chunks are consumed slower than the ring delivers;
others {0,1}/{2,3}; weight panels re-read per phase -- cheap in fp8) so the
LN/SiLU/quant/transpose tail of one phase's tiles overlaps the opposite
phase's matmuls. Engine queues are in-order, so AR123-gated prologue work
(scales, L1-3 quant) is emitted via hooks behind L0's main-pass work to avoid
head-of-line blocking of L0's evictions and tails.
"""

import numpy as np

import concourse.bass as bass
import concourse.mybir as mybir
import concourse.tile as tile
from concourse import bacc
from concourse import bass_isa
from concourse.bass_utils import run_bass_kernel_spmd

F32 = mybir.dt.float32
BF16 = mybir.dt.bfloat16
FP8 = mybir.dt.float8e4
AF = mybir.ActivationFunctionType
OP = mybir.AluOpType

N_CORES = 8
P = 128
OBW = 512            # output block width (one PSUM bank of fp32)
CH = P * OBW         # elements per [128, 512] weight chunk
RUN = 8              # abs/quant pass block = [128, RUN*512]
MAGIC = 12582912.0   # 1.5 * 2**23: fp32 round-to-nearest-even trick
EPS = 1e-5

FULL_CFG = dict(B=4096, D0=1024, H=4096, OBINS=1000)


def _plan(cfg):
    """Static per-layer plan.

    Rank r owns ic-chunks [r*q, (r+1)*q) of every ob of every layer. The
    per-rank weight stream (wsh / stage) is ordered [layer][ob-group g]
    with each (layer, g) block stored [128, opg*q*512] partition-major.
    AllGather of block (l, g) yields complete weights for obs of group g.
    """
    B, D0, H, OBINS = cfg["B"], cfg["D0"], cfg["H"], cfg["OBINS"]
    o3_real = 2 * OBINS
    o3_pad = ((o3_real + OBW - 1) // OBW) * OBW
    dims = [
        dict(din=D0, dout=H, dreal=H),
        dict(din=H, dout=H, dreal=H),
        dict(din=H, dout=H, dreal=H),
        dict(din=H, dout=o3_pad, dreal=o3_real),
    ]
    numels = [H * D0, H * H, H * H, o3_real * H]  # real numels for mean|W|
    layers = []
    woff = 0
    for li, d in enumerate(dims):
        n_ic = d["din"] // P
        n_ob = d["dout"] // OBW
        assert n_ic % N_CORES == 0, (li, n_ic)
        q = n_ic // N_CORES
        n_g = min((1, 4, 2, 2)[li], n_ob)
        assert n_ob % n_g == 0
        opg = n_ob // n_g           # obs per AllGather group
        gw = opg * q * OBW          # per-rank (l, g) block width (cols)
        layers.append(dict(
            li=li, din=d["din"], dout=d["dout"], dreal=d["dreal"],
            n_ic=n_ic, n_ob=n_ob, q=q, n_g=n_g, opg=opg, gw=gw,
            numel=numels[li], woff=woff,
            ob_w=[min(OBW, d["dreal"] - ob * OBW) for ob in range(n_ob)],
        ))
        woff += n_g * P * gw
    per_rank = woff                 # fp32 elements per rank in wsh
    b_core = B // N_CORES
    assert b_core % P == 0
    return layers, per_rank, b_core // P


def _rsqrt_newton(nc, pool, v, n_iter=3):
    """istd = 1/sqrt(v) for v [128,1] fp32 (v > 0), pure-DVE Newton."""
    seed = pool.tile([P, 1], F32, tag="rs_seed", name="rs_seed")
    seed_i32 = seed[:].bitcast(mybir.dt.int32)
    v_i32 = v.bitcast(mybir.dt.int32)
    nc.vector.tensor_scalar(seed_i32[:], v_i32[:], -0.5,
                            float(0x5F370000), OP.mult, OP.add)
    y = seed
    t1 = pool.tile([P, 1], F32, tag="rs_t1", name="rs_t1")
    t2 = pool.tile([P, 1], F32, tag="rs_t2", name="rs_t2")
    for _ in range(n_iter):
        nc.vector.tensor_tensor(t1[:], y[:], y[:], OP.mult)
        nc.vector.tensor_tensor(t2[:], t1[:], v, OP.mult)
        nc.vector.tensor_scalar(t1[:], t2[:], -0.5, 1.5, OP.mult, OP.add)
        nc.vector.tensor_tensor(y[:], y[:], t1[:], OP.mult)
    return y


def build(cfg):
    layers, per_rank, T = _plan(cfg)
    nc = bacc.Bacc("TRN2", target_bir_lowering=False, debug=False,
                   num_devices=N_CORES)

    D0, OBINS = cfg["D0"], cfg["OBINS"]
    b_core = T * P
    n_ic0 = layers[0]["n_ic"]

    xs = nc.dram_tensor("xs", [b_core, D0], F32, kind="ExternalInput")
    wsh = nc.dram_tensor("wsh", [per_rank], F32, kind="ExternalInput")
    mz_out = nc.dram_tensor("mz", [b_core, OBINS], F32, kind="ExternalOutput")
    ii_out = nc.dram_tensor("ii", [b_core, OBINS], F32, kind="ExternalOutput")

    # abs pass streams RUN8-wide blocks; quant pass re-reads RUN4-wide
    def _mkruns(rw):
        rr = []
        for L in layers:
            for g in range(L["n_g"]):
                c0 = 0
                while c0 < L["gw"]:
                    w = min(rw * OBW, L["gw"] - c0)
                    rr.append((L["li"], g, c0, w))
                    c0 += w
        return rr

    runs8 = _mkruns(RUN)
    runs4 = _mkruns(RUN // 2)
    lcols = {li: [ri for ri, r in enumerate(runs8) if r[0] == li]
             for li in range(4)}
    n_runs = len(runs8)

    def wsh_off(li, g):
        L = layers[li]
        return L["woff"] + g * P * L["gw"]

    with tile.TileContext(nc) as tc:
        with (
            tc.tile_pool(name="ybig", bufs=3) as ypool,     # [128,4096] f32
            tc.tile_pool(name="wr", bufs=2) as wrpool,      # [128,4096] f32
            tc.tile_pool(name="wq", bufs=2) as wqpool,      # [128,2048] f32
            tc.tile_pool(name="xqT", bufs=5) as xqTpool,    # [128,32,128] bf16
            tc.tile_pool(name="xqT0", bufs=4) as xqT0pool,  # [128,n_ic0,128]
            tc.tile_pool(name="wp", bufs=2) as wpool,       # [128,n_ic,512] fp8
            tc.tile_pool(name="xqn", bufs=2) as xqnpool,    # [128,4096] bf16
            tc.tile_pool(name="qo", bufs=2) as qopool,      # [128,2048] fp8
            tc.tile_pool(name="sg", bufs=2) as sgpool,      # [128,512] f32
            tc.tile_pool(name="u", bufs=2) as upool,        # [128,512] f32
            tc.tile_pool(name="small", bufs=1) as small,
            tc.tile_pool(name="psum", bufs=8, space="PSUM") as psum,
            tc.tile_pool(name="dram", bufs=1, space="DRAM") as dram,
        ):
            # ---------------- DRAM scratch ----------------
            stage = {}
            image = {}
            for L in layers:
                li = L["li"]
                for g in range(L["n_g"]):
                    sz = P * L["gw"]
                    stage[(li, g)] = dram.tile([sz], FP8, tag=f"st{li}_{g}",
                                               name=f"st{li}_{g}")
                    image[(li, g)] = dram.tile([N_CORES * sz], FP8,
                                               tag=f"im{li}_{g}",
                                               name=f"im{li}_{g}",
                                               addr_space="Shared")
            ar_in0 = dram.tile([P, 1], F32, tag="ar_in0", name="ar_in0")
            ar_out0 = dram.tile([P, 1], F32, tag="ar_out0", name="ar_out0",
                                addr_space="Shared")
            ar_in123 = dram.tile([P, 3], F32, tag="ar_in123", name="ar_in123")
            ar_out123 = dram.tile([P, 3], F32, tag="ar_out123",
                                  name="ar_out123", addr_space="Shared")

            # shared small tiles
            invn = small.tile([P, 4], F32, tag="invn", name="invn")
            for li, L in enumerate(layers):
                nc.any.memset(invn[:, li:li + 1], 1.0 / L["numel"])
            mwb4 = small.tile([P, 4], F32, tag="mwb4", name="mwb4")
            swb4 = small.tile([P, 4], F32, tag="swb4", name="swb4")
            pmsr = small.tile([P, 4], F32, tag="pmsr", name="pmsr")
            mwb = [mwb4[:, li:li + 1] for li in range(4)]
            swb = [swb4[:, li:li + 1] for li in range(4)]
            pm = small.tile([P, 4], F32, tag="pm", name="pm")
            pms = small.tile([P, 4], F32, tag="pms", name="pms")
            partials = small.tile([P, n_runs], F32, tag="partials",
                                  name="partials")

            def _abs_issue(lis, dma):
                out = []
                for ri, (li, g, c0, w) in enumerate(runs8):
                    if li not in lis:
                        continue
                    off = wsh_off(li, g) + c0 * P
                    wrun = wrpool.tile([P, RUN * OBW], F32, tag="wr",
                                       name=f"wrB{ri}")
                    dma(wrun[:, :w], wsh[off:off + P * w].rearrange(
                        "(p f) -> p f", p=P))
                    out.append((ri, wrun, w))
                return out

            def _abs_compute(issued, dve_only=False):
                for k, (ri, wrun, w) in enumerate(issued):
                    if dve_only or k % 2 == 0:
                        nc.vector.tensor_reduce(partials[:, ri:ri + 1],
                                                wrun[:, :w],
                                                mybir.AxisListType.X,
                                                OP.add,
                                                apply_absolute_value=True)
                    else:
                        nc.scalar.activation(wrun[:, :w], wrun[:, :w], AF.Abs,
                                             bias=0.0, scale=1.0,
                                             accum_out=partials[:, ri:ri + 1])

            def _pack_partials(lis, col0):
                for i, li in enumerate(lis):
                    cols = lcols[li]
                    assert cols == list(range(cols[0], cols[-1] + 1))
                    nc.vector.tensor_reduce(pm[:, col0 + i:col0 + i + 1],
                                            partials[:, cols[0]:cols[-1] + 1],
                                            mybir.AxisListType.X, OP.add)

            def _scale_par(col0, ncols):
                sl = slice(col0, col0 + ncols)
                nc.gpsimd.partition_all_reduce(pmsr[:, sl], pms[:, sl],
                                               channels=P,
                                               reduce_op=bass_isa.ReduceOp.add)

            def _scale_dve(col0, ncols):
                sl = slice(col0, col0 + ncols)
                nc.vector.tensor_tensor(mwb4[:, sl], pmsr[:, sl],
                                        invn[:, sl], OP.mult)
                nc.vector.tensor_scalar(mwb4[:, sl], mwb4[:, sl], float(EPS),
                                        None, OP.max)
                nc.vector.reciprocal(swb4[:, sl], mwb4[:, sl])

            def _scale_post(col0, ncols):
                _scale_par(col0, ncols)
                _scale_dve(col0, ncols)

            def _quant_issue(lis, dma):
                out = []
                for ri, (li, g, c0, w) in enumerate(runs4):
                    if li not in lis:
                        continue
                    off = wsh_off(li, g) + c0 * P
                    wrun = wqpool.tile([P, RUN // 2 * OBW], F32, tag="wq",
                                       name=f"wrC{ri}")
                    dma(wrun[:, :w], wsh[off:off + P * w].rearrange(
                        "(p f) -> p f", p=P))
                    out.append((ri, wrun, w))
                return out

            def _quant_compute(issued, dve_only=False):
                for ri, wrun, w in issued:
                    li, g, c0, _ = runs4[ri]
                    if dve_only:
                        nc.vector.tensor_scalar(wrun[:, :w], wrun[:, :w],
                                                swb[li], MAGIC,
                                                OP.mult, OP.add)
                    else:
                        nc.scalar.activation(wrun[:, :w], wrun[:, :w],
                                             AF.Copy, bias=MAGIC,
                                             scale=swb[li])
                    nc.vector.tensor_scalar(wrun[:, :w], wrun[:, :w], MAGIC,
                                            1.0, OP.subtract, OP.min)
                    qblk = qopool.tile([P, RUN // 2 * OBW], FP8, tag="qo",
                                       name=f"qo{ri}")
                    nc.vector.tensor_scalar(qblk[:, :w], wrun[:, :w], -1.0,
                                            None, OP.max)
                    soff = c0 * P
                    nc.gpsimd.dma_start(
                        stage[(li, g)][soff:soff + P * w].rearrange(
                            "(p f) -> p f", p=P),
                        qblk[:, :w])
                    if c0 + w == layers[li]["gw"]:
                        # bf16-typed APs double the per-descriptor payload of
                        # the AllGather (byte-identical data)
                        nc.gpsimd.collective_compute(
                            "AllGather", OP.bypass,
                            ins=[stage[(li, g)][:].bitcast(BF16)],
                            outs=[image[(li, g)][:].bitcast(BF16)],
                            replica_groups=[list(range(N_CORES))])

            # S1: prewarm gpsimd DSP lib; L0/L1 abs reads + x reads in flight
            prew = small.tile([P, 4], F32, tag="prew", name="prew")
            nc.any.memset(prew[:], 0)
            nc.gpsimd.partition_all_reduce(prew[:], prew[:], channels=P,
                                           reduce_op=bass_isa.ReduceOp.add)
            issued0 = _abs_issue([0], nc.sync.dma_start)
            xts = []
            for t in range(T):
                xt = ypool.tile([P, D0], F32, tag="y", name=f"xt{t}")
                nc.sync.dma_start(xt[:], xs[t * P:(t + 1) * P, :])
                xts.append(xt)
            issued1 = _abs_issue([1], nc.scalar.dma_start)

            # S2: L0 sums -> AR0 launched immediately (rides the barrier)
            _abs_compute(issued0)
            _pack_partials([0], 0)
            nc.gpsimd.dma_start(ar_in0[:], pm[:, 0:1])
            nc.gpsimd.collective_compute(
                "AllReduce", OP.add,
                ins=[ar_in0.opt()], outs=[ar_out0.opt()],
                replica_groups=[list(range(N_CORES))])
            # S3: L1 sums
            _abs_compute(issued1)

            # S3.5: input activation quant (x already in flight)
            xqT_cur = []
            am0s = []
            for t in range(T):
                xt = xts[t]
                am = small.tile([P, 1], F32, tag=f"am0_{t}", name=f"am0_{t}")
                nc.vector.tensor_reduce(am[:], xt[:], mybir.AxisListType.X,
                                        OP.max, apply_absolute_value=True)
                nc.vector.tensor_scalar(am[:], am[:], float(EPS), None, OP.max)
                sc = small.tile([P, 1], F32, tag=f"s0_{t}", name=f"s0_{t}")
                nc.vector.tensor_scalar(sc[:], am[:], 1.0 / 127.0, None,
                                        OP.mult)
                nc.vector.reciprocal(sc[:], sc[:])
                xq0 = xqnpool.tile([P, D0], BF16, tag="xqn", name=f"xq0_{t}")
                for ch in range(D0 // OBW):
                    uu = upool.tile([P, OBW], F32, tag="u", name=f"u0_{t}_{ch}")
                    nc.scalar.activation(uu[:], xt[:, ch * OBW:(ch + 1) * OBW],
                                         AF.Copy, bias=MAGIC, scale=sc[:])
                    nc.vector.tensor_scalar(xq0[:, ch * OBW:(ch + 1) * OBW],
                                            uu[:], MAGIC, None, OP.subtract)
                xqT0 = xqT0pool.tile([P, n_ic0, P], BF16, tag="xqT0",
                                     name=f"xqT0_{t}")
                nc.scalar.dma_start_transpose(xqT0[:], xq0[:])
                xqT_cur.append(xqT0)
                am0s.append(am)

            # S4: AR0 readback + L0 scale
            nc.sync.dma_start(pms[:, 0:1], ar_out0[:])
            _scale_post(0, 1)
            # S5: L2/L3 abs reads (after AR0 on the gpsimd queue)
            issued23 = _abs_issue([2, 3], nc.gpsimd.dma_start)
            # S7: quant L0 + its AllGather
            q0 = _quant_issue([0], nc.sync.dma_start)
            _quant_compute(q0)
            # S8: L1 quant reads (data-independent, fire early)
            q1 = _quant_issue([1], nc.sync.dma_start)
            c_cur = []
            for t in range(T):
                c0t = small.tile([P, 1], F32, tag=f"c0_{t}", name=f"c0_{t}")
                nc.vector.scalar_tensor_tensor(c0t[:], am0s[t][:], 1.0 / 127.0,
                                               mwb[0], OP.mult, OP.mult)
                c_cur.append(c0t)
            # S9: L2/L3 sums (DVE-only: the scalar queue must stay clear
            # for quant-L0 -> AG-L0 and then L0's evictions)
            _abs_compute(issued23, dve_only=True)
            # S10: AR123 + L1 quant (pure DVE so L0 evictions aren't blocked
            # behind it on the scalar queue) + L1 AllGather chunks
            _pack_partials([1, 2, 3], 1)
            nc.gpsimd.dma_start(ar_in123[:], pm[:, 1:4])
            nc.gpsimd.collective_compute(
                "AllReduce", OP.add,
                ins=[ar_in123.opt()], outs=[ar_out123.opt()],
                replica_groups=[list(range(N_CORES))])
            nc.gpsimd.dma_start(pms[:, 1:4], ar_out123[:])
            _scale_par(1, 3)
            _scale_dve(1, 3)
            _quant_compute(q1[:1], dve_only=True)
            q23 = [None]
            q23[0] = _quant_issue([2, 3], nc.gpsimd.dma_start)
            _quant_compute(q1[1:], dve_only=True)

            # ---------------- Stage D: main pass ----------------
            for L in layers:
                if T != 4:
                    phases = [tuple(range(T))]
                elif L["li"] == 1:
                    # L1's AllGather chunks trail the matmuls; a 3-tile first
                    # phase consumes each chunk slower than the ring delivers
                    phases = [(0, 1, 2), (3,)]
                else:
                    phases = [(0, 1), (2, 3)]
                li, n_ic, n_ob, q = L["li"], L["n_ic"], L["n_ob"], L["q"]
                opg, gw = L["opg"], L["gw"]
                dout, dreal = L["dout"], L["dreal"]
                is_last = (li == 3)
                S = P * gw  # per-rank image block size (elements)

                ys = {}
                bns = {}
                xqT_next = {}
                c_next = {}

                for ph, ptiles in enumerate(phases):
                    for ob in range(n_ob):
                        g, obl = divmod(ob, opg)
                        ow = L["ob_w"][ob]
                        wp = wpool.tile([P, n_ic, OBW], FP8, tag="wp",
                                        name=f"wp{li}_{ph}_{ob}")
                        img = image[(li, g)]
                        for rb in range(N_CORES):
                            src = img[rb * S:(rb + 1) * S].rearrange(
                                "(p f) -> p f", p=P)
                            nc.sync.dma_start(
                                wp[:, rb * q:(rb + 1) * q, :],
                                src[:, obl * q * OBW:(obl + 1) * q * OBW])
                        for t in ptiles:
                            if ob == 0:
                                ys[t] = ypool.tile([P, dreal], F32, tag="y",
                                                   name=f"y{li}_{t}")
                                if not is_last:
                                    bns[t] = small.tile(
                                        [P, n_ob * 6], F32, tag=f"bn{t}",
                                        name=f"bn{li}_{t}")
                            ps = psum.tile([P, OBW], F32, tag="ps",
                                           name=f"ps{li}_{ob}_{t}")
                            for c in range(n_ic):
                                nc.tensor.matmul(
                                    ps[:], xqT_cur[t][:, c, :], wp[:, c, :],
                                    start=(c == 0), stop=(c == n_ic - 1))
                            dst = ys[t][:, ob * OBW:ob * OBW + ow]
                            if not is_last:
                                nc.scalar.activation(dst, ps[:, :ow], AF.Copy,
                                                     bias=0.0,
                                                     scale=c_cur[t][:])
                                nc.vector.bn_stats(
                                    bns[t][:, ob * 6:(ob + 1) * 6], dst)
                            else:
                                nc.scalar.activation(dst, ps[:, :ow],
                                                     AF.Sigmoid, bias=0.0,
                                                     scale=c_cur[t][:])

                    if li == 1 and ph == 0:
                        # L2/L3 weight quant + AllGathers: before the phase
                        # tails so AG-L2a triggers as early as possible
                        _quant_compute(q23[0])
                    # ---- phase tail ----
                    for t in ptiles:
                        if is_last:
                            nc.vector.tensor_scalar(ys[t][:, 0:OBINS],
                                                    ys[t][:, 0:OBINS],
                                                    float(OBINS - 1), 1.0,
                                                    OP.mult, OP.add)
                            nc.scalar.dma_start(mz_out[t * P:(t + 1) * P, :],
                                                ys[t][:, 0:OBINS])
                            nc.vector.tensor_scalar(ys[t][:, OBINS:2 * OBINS],
                                                    ys[t][:, OBINS:2 * OBINS],
                                                    100.0, None, OP.mult)
                            nc.scalar.dma_start(ii_out[t * P:(t + 1) * P, :],
                                                ys[t][:, OBINS:2 * OBINS])
                            continue
                        # LN + SiLU + act quant + transpose
                        n_ic_next = layers[li + 1]["n_ic"]
                        mv = small.tile([P, 2], F32, tag=f"mv{t}",
                                        name=f"mv{li}_{t}")
                        nc.vector.bn_aggr(mv[:], bns[t][:])
                        v = small.tile([P, 1], F32, tag=f"vvar{t}",
                                       name=f"v{li}_{t}")
                        nc.vector.tensor_scalar(v[:], mv[:, 1:2], float(EPS),
                                                None, OP.add)
                        istd = _rsqrt_newton(nc, small, v[:])
                        nmi = small.tile([P, 1], F32, tag=f"nmi{t}",
                                         name=f"nmi{li}_{t}")
                        nc.vector.scalar_tensor_tensor(nmi[:], mv[:, 0:1],
                                                       -1.0, istd[:],
                                                       OP.mult, OP.mult)
                        nc.scalar.activation(ys[t][:], ys[t][:], AF.Identity,
                                             bias=nmi[:], scale=istd[:])
                        amsl = small.tile([P, 8], F32, tag=f"amsl{t}",
                                          name=f"amsl{li}_{t}")
                        n_chk = dout // OBW
                        for ch in range(n_chk):
                            sl = ys[t][:, ch * OBW:(ch + 1) * OBW]
                            sg = sgpool.tile([P, OBW], F32, tag="sg",
                                             name=f"sg{li}_{t}_{ch}")
                            nc.scalar.activation(sg[:], sl, AF.Tanh,
                                                 bias=0.0, scale=0.5)
                            nc.vector.tensor_scalar(sg[:], sg[:], 0.5, 0.5,
                                                    OP.mult, OP.add)
                            nc.vector.tensor_tensor(sl, sl, sg[:], OP.mult)
                            nc.vector.tensor_reduce(amsl[:, ch:ch + 1], sl,
                                                    mybir.AxisListType.X,
                                                    OP.max,
                                                    apply_absolute_value=True)
                        am = small.tile([P, 1], F32, tag=f"amn{t}",
                                        name=f"am{li}_{t}")
                        nc.vector.tensor_reduce(am[:], amsl[:, :n_chk],
                                                mybir.AxisListType.X, OP.max)
                        nc.vector.tensor_scalar(am[:], am[:], float(EPS),
                                                None, OP.max)
                        sc = small.tile([P, 1], F32, tag=f"scn{t}",
                                        name=f"sc{li}_{t}")
                        nc.vector.tensor_scalar(sc[:], am[:], 1.0 / 127.0,
                                                None, OP.mult)
                        nc.vector.reciprocal(sc[:], sc[:])
                        cn = small.tile([P, 1], F32, tag=f"cn{t}",
                                        name=f"c{li + 1}_{t}")
                        nc.vector.scalar_tensor_tensor(cn[:], am[:],
                                                       1.0 / 127.0,
                                                       mwb[li + 1],
                                                       OP.mult, OP.mult)
                        c_next[t] = cn
                        xqn = xqnpool.tile([P, dout], BF16, tag="xqn",
                                           name=f"xqn{li}_{t}")
                        for ch in range(n_chk):
                            uu = upool.tile([P, OBW], F32, tag="u",
                                            name=f"ur{li}_{t}_{ch}")
                            nc.scalar.activation(
                                uu[:], ys[t][:, ch * OBW:(ch + 1) * OBW],
                                AF.Copy, bias=MAGIC, scale=sc[:])
                            nc.vector.tensor_scalar(
                                xqn[:, ch * OBW:(ch + 1) * OBW],
                                uu[:], MAGIC, None, OP.subtract)
                        xT = xqTpool.tile([P, n_ic_next, P], BF16, tag="xqT",
                                          name=f"xT{li}_{t}")
                        nc.scalar.dma_start_transpose(xT[:], xqn[:])
                        xqT_next[t] = xT

                if not is_last:
                    xqT_cur = [xqT_next[t] for t in range(T)]
                    c_cur = [c_next[t] for t in range(T)]

    nc.compile()
    return nc


def prepare_inputs(cfg, x, W0, W1, W2, W3):
    """Host-side sharding (pure layout, no arithmetic): per-core input maps.

    wsh layout per rank r: for layer li, for ob-group g, a [128, gw] fp32
    block (partition-major flat) whose cols are [ob-local][j][o] with
    j in [0, q): value = W_li^T[(r*q+j)*128 + p, ob*512 + o].
    """
    layers, per_rank, T = _plan(cfg)
    b_core = T * P
    Ws = [np.asarray(W0), np.asarray(W1), np.asarray(W2), np.asarray(W3)]
    WTs = []
    for L, W in zip(layers, Ws):
        WT = np.zeros((L["din"], L["dout"]), dtype=np.float32)
        WT[:, :L["dreal"]] = W.T
        WTs.append(WT)

    shards = [np.empty(per_rank, dtype=np.float32) for _ in range(N_CORES)]
    for L in layers:
        li, q, opg, gw = L["li"], L["q"], L["opg"], L["gw"]
        WT = WTs[li]
        for r in range(N_CORES):
            for g in range(L["n_g"]):
                blk = np.empty((P, gw), dtype=np.float32)
                for obl in range(opg):
                    ob = g * opg + obl
                    for j in range(q):
                        ic = r * q + j
                        blk[:, (obl * q + j) * OBW:(obl * q + j + 1) * OBW] = \
                            WT[ic * P:(ic + 1) * P, ob * OBW:(ob + 1) * OBW]
                off = L["woff"] + g * P * gw
                shards[r][off:off + P * gw] = blk.reshape(-1)
    x = np.asarray(x, dtype=np.float32)
    in_maps = []
    for r in range(N_CORES):
        in_maps.append(dict(
            xs=np.ascontiguousarray(x[r * b_core:(r + 1) * b_core]),
            wsh=shards[r],
        ))
    return in_maps


_NC_CACHE = {}


def _get_nc(cfg_key):
    if cfg_key not in _NC_CACHE:
        _NC_CACHE[cfg_key] = build(dict(cfg_key))
    return _NC_CACHE[cfg_key]


def run(cfg, x, W0, W1, W2, W3, trace=False):
    layers, per_rank, T = _plan(cfg)
    b_core = T * P
    nc = _get_nc(tuple(sorted(cfg.items())))
    in_maps = prepare_inputs(cfg, x, W0, W1, W2, W3)
    res = run_bass_kernel_spmd(nc, in_maps, core_ids=list(range(N_CORES)),
                               trace=trace)
    mz = np.concatenate([res.results[r]["mz"] for r in range(N_CORES)], axis=0)
    ii = np.concatenate([res.results[r]["ii"] for r in range(N_CORES)], axis=0)
    return (mz, ii), res


def kernel(x, W0, W1, W2, W3, g0, b0, g1, b1, g2, b2):
    """Full-input entry point. g/b are identity (ones/zeros) in this problem's
    setup; LayerNorm affine is a no-op and is validated here."""
    for g in (g0, g1, g2):
        assert np.allclose(np.asarray(g), 1.0), "non-identity LN gain unsupported"
    for b in (b0, b1, b2):
        assert np.allclose(np.asarray(b), 0.0), "non-zero LN bias unsupported"
    (mz, ii), _ = run(FULL_CFG, x, W0, W1, W2, W3, trace=False)
    return (mz, ii)


# revision 26
# speedup vs baseline: 1.0504x; 1.0504x over previous
"""BitNet decoder MLP on 8 Trainium2 NeuronCores (Bass/Tile).

Strategy: data-parallel over batch (512 rows/core). Weights are ternary-
quantized cooperatively: each core owns the ic-slice [r*q, (r+1)*q) of every
output block of every layer. Two AllReduces supply the |W| means (L0's needs
only 2MB/core of reads, so it rides right behind the kernel-entry barrier and
unblocks L0's quant+AllGather while the L1-3 abs stream is still in flight);
each core quantizes its slice to an fp8e4 {-1,0,1} image, and per-layer
ob-group-chunked AllGathers (1/4/2/2 chunks) ship complete output-column
groups in consumption order so layer-l matmuls start as soon as their first
ob-group lands.

All matmul arithmetic is exact: activations are int8-valued bf16 (stationary),
weights are {-1,0,1} fp8e4 (moving) -- the mixed-dtype matmul is bit-exact on
TRN2 hardware and halves both AllGather bytes and main-pass weight DMA vs
bf16. Accumulation is fp32 in PSUM. Per-row dequant scales fold into the
PSUM-eviction pass on the scalar engine; LayerNorm+SiLU run as fused
scalar-engine passes (sigmoid via the tanh table); rsqrt via Newton on the
vector engine; rounding via the fp32 magic-number trick (round-half-even).

The main pass runs each layer in two batch-tile phases (L1: {0,1,2}/{3} so
its trailing AllGather chunks are consumed slower than the ring delivers;
others {0,1}/{2,3}; weight panels re-read per phase -- cheap in fp8) so the
LN/SiLU/quant/transpose tail of one phase's tiles overlaps the opposite
phase's matmuls. Engine queues are in-order, so AR123-gated prologue work
(scales, L1-3 quant) is emitted via hooks behind L0's main-pass work to avoid
head-of-line blocking of L0's evictions and tails.
"""

import numpy as np

import concourse.bass as bass
import concourse.mybir as mybir
import concourse.tile as tile
from concourse import bacc
from concourse import bass_isa
from concourse.bass_utils import run_bass_kernel_spmd

F32 = mybir.dt.float32
BF16 = mybir.dt.bfloat16
FP8 = mybir.dt.float8e4
AF = mybir.ActivationFunctionType
OP = mybir.AluOpType

N_CORES = 8
P = 128
OBW = 512            # output block width (one PSUM bank of fp32)
CH = P * OBW         # elements per [128, 512] weight chunk
RUN = 8              # abs/quant pass block = [128, RUN*512]
MAGIC = 12582912.0   # 1.5 * 2**23: fp32 round-to-nearest-even trick
EPS = 1e-5

FULL_CFG = dict(B=4096, D0=1024, H=4096, OBINS=1000)


def _plan(cfg):
    """Static per-layer plan.

    Rank r owns ic-chunks [r*q, (r+1)*q) of every ob of every layer. The
    per-rank weight stream (wsh / stage) is ordered [layer][ob-group g]
    with each (layer, g) block stored [128, opg*q*512] partition-major.
    AllGather of block (l, g) yields complete weights for obs of group g.
    """
    B, D0, H, OBINS = cfg["B"], cfg["D0"], cfg["H"], cfg["OBINS"]
    o3_real = 2 * OBINS
    o3_pad = ((o3_real + OBW - 1) // OBW) * OBW
    dims = [
        dict(din=D0, dout=H, dreal=H),
        dict(din=H, dout=H, dreal=H),
        dict(din=H, dout=H, dreal=H),
        dict(din=H, dout=o3_pad, dreal=o3_real),
    ]
    numels = [H * D0, H * H, H * H, o3_real * H]  # real numels for mean|W|
    layers = []
    woff = 0
    for li, d in enumerate(dims):
        n_ic = d["din"] // P
        n_ob = d["dout"] // OBW
        assert n_ic % N_CORES == 0, (li, n_ic)
        q = n_ic // N_CORES
        n_g = min((1, 4, 4, 2)[li], n_ob)
        assert n_ob % n_g == 0
        opg = n_ob // n_g           # obs per AllGather group
        gw = opg * q * OBW          # per-rank (l, g) block width (cols)
        layers.append(dict(
            li=li, din=d["din"], dout=d["dout"], dreal=d["dreal"],
            n_ic=n_ic, n_ob=n_ob, q=q, n_g=n_g, opg=opg, gw=gw,
            numel=numels[li], woff=woff,
            ob_w=[min(OBW, d["dreal"] - ob * OBW) for ob in range(n_ob)],
        ))
        woff += n_g * P * gw
    per_rank = woff                 # fp32 elements per rank in wsh
    b_core = B // N_CORES
    assert b_core % P == 0
    return layers, per_rank, b_core // P


def _rsqrt_newton(nc, pool, v, n_iter=3):
    """istd = 1/sqrt(v) for v [128,1] fp32 (v > 0), pure-DVE Newton."""
    seed = pool.tile([P, 1], F32, tag="rs_seed", name="rs_seed")
    seed_i32 = seed[:].bitcast(mybir.dt.int32)
    v_i32 = v.bitcast(mybir.dt.int32)
    nc.vector.tensor_scalar(seed_i32[:], v_i32[:], -0.5,
                            float(0x5F370000), OP.mult, OP.add)
    y = seed
    t1 = pool.tile([P, 1], F32, tag="rs_t1", name="rs_t1")
    t2 = pool.tile([P, 1], F32, tag="rs_t2", name="rs_t2")
    for _ in range(n_iter):
        nc.vector.tensor_tensor(t1[:], y[:], y[:], OP.mult)
        nc.vector.tensor_tensor(t2[:], t1[:], v, OP.mult)
        nc.vector.tensor_scalar(t1[:], t2[:], -0.5, 1.5, OP.mult, OP.add)
        nc.vector.tensor_tensor(y[:], y[:], t1[:], OP.mult)
    return y


def build(cfg):
    layers, per_rank, T = _plan(cfg)
    nc = bacc.Bacc("TRN2", target_bir_lowering=False, debug=False,
                   num_devices=N_CORES)

    D0, OBINS = cfg["D0"], cfg["OBINS"]
    b_core = T * P
    n_ic0 = layers[0]["n_ic"]

    xs = nc.dram_tensor("xs", [b_core, D0], F32, kind="ExternalInput")
    wsh = nc.dram_tensor("wsh", [per_rank], F32, kind="ExternalInput")
    mz_out = nc.dram_tensor("mz", [b_core, OBINS], F32, kind="ExternalOutput")
    ii_out = nc.dram_tensor("ii", [b_core, OBINS], F32, kind="ExternalOutput")

    # abs pass streams RUN8-wide blocks; quant pass re-reads RUN4-wide
    def _mkruns(rw):
        rr = []
        for L in layers:
            for g in range(L["n_g"]):
                c0 = 0
                while c0 < L["gw"]:
                    w = min(rw * OBW, L["gw"] - c0)
                    rr.append((L["li"], g, c0, w))
                    c0 += w
        return rr

    runs8 = _mkruns(RUN)
    runs4 = _mkruns(RUN // 2)
    lcols = {li: [ri for ri, r in enumerate(runs8) if r[0] == li]
             for li in range(4)}
    n_runs = len(runs8)

    def wsh_off(li, g):
        L = layers[li]
        return L["woff"] + g * P * L["gw"]

    with tile.TileContext(nc) as tc:
        with (
            tc.tile_pool(name="ybig", bufs=3) as ypool,     # [128,4096] f32
            tc.tile_pool(name="wr", bufs=2) as wrpool,      # [128,4096] f32
            tc.tile_pool(name="wq", bufs=2) as wqpool,      # [128,2048] f32
            tc.tile_pool(name="xqT", bufs=5) as xqTpool,    # [128,32,128] bf16
            tc.tile_pool(name="xqT0", bufs=4) as xqT0pool,  # [128,n_ic0,128]
            tc.tile_pool(name="wp", bufs=2) as wpool,       # [128,n_ic,512] fp8
            tc.tile_pool(name="xqn", bufs=2) as xqnpool,    # [128,4096] bf16
            tc.tile_pool(name="qo", bufs=2) as qopool,      # [128,2048] fp8
            tc.tile_pool(name="sg", bufs=2) as sgpool,      # [128,512] f32
            tc.tile_pool(name="u", bufs=2) as upool,        # [128,512] f32
            tc.tile_pool(name="small", bufs=1) as small,
            tc.tile_pool(name="psum", bufs=8, space="PSUM") as psum,
            tc.tile_pool(name="dram", bufs=1, space="DRAM") as dram,
        ):
            # ---------------- DRAM scratch ----------------
            stage = {}
            image = {}
            for L in layers:
                li = L["li"]
                for g in range(L["n_g"]):
                    sz = P * L["gw"]
                    stage[(li, g)] = dram.tile([sz], FP8, tag=f"st{li}_{g}",
                                               name=f"st{li}_{g}")
                    image[(li, g)] = dram.tile([N_CORES * sz], FP8,
                                               tag=f"im{li}_{g}",
                                               name=f"im{li}_{g}",
                                               addr_space="Shared")
            ar_in0 = dram.tile([P, 1], F32, tag="ar_in0", name="ar_in0")
            ar_out0 = dram.tile([P, 1], F32, tag="ar_out0", name="ar_out0",
                                addr_space="Shared")
            ar_in123 = dram.tile([P, 3], F32, tag="ar_in123", name="ar_in123")
            ar_out123 = dram.tile([P, 3], F32, tag="ar_out123",
                                  name="ar_out123", addr_space="Shared")

            # shared small tiles
            invn = small.tile([P, 4], F32, tag="invn", name="invn")
            for li, L in enumerate(layers):
                nc.any.memset(invn[:, li:li + 1], 1.0 / L["numel"])
            mwb4 = small.tile([P, 4], F32, tag="mwb4", name="mwb4")
            swb4 = small.tile([P, 4], F32, tag="swb4", name="swb4")
            pmsr = small.tile([P, 4], F32, tag="pmsr", name="pmsr")
            mwb = [mwb4[:, li:li + 1] for li in range(4)]
            swb = [swb4[:, li:li + 1] for li in range(4)]
            pm = small.tile([P, 4], F32, tag="pm", name="pm")
            pms = small.tile([P, 4], F32, tag="pms", name="pms")
            partials = small.tile([P, n_runs], F32, tag="partials",
                                  name="partials")

            def _abs_issue(lis, dma):
                out = []
                for ri, (li, g, c0, w) in enumerate(runs8):
                    if li not in lis:
                        continue
                    off = wsh_off(li, g) + c0 * P
                    wrun = wrpool.tile([P, RUN * OBW], F32, tag="wr",
                                       name=f"wrB{ri}")
                    dma(wrun[:, :w], wsh[off:off + P * w].rearrange(
                        "(p f) -> p f", p=P))
                    out.append((ri, wrun, w))
                return out

            def _abs_compute(issued, dve_only=False):
                for k, (ri, wrun, w) in enumerate(issued):
                    if dve_only or k % 2 == 0:
                        nc.vector.tensor_reduce(partials[:, ri:ri + 1],
                                                wrun[:, :w],
                                                mybir.AxisListType.X,
                                                OP.add,
                                                apply_absolute_value=True)
                    else:
                        nc.scalar.activation(wrun[:, :w], wrun[:, :w], AF.Abs,
                                             bias=0.0, scale=1.0,
                                             accum_out=partials[:, ri:ri + 1])

            def _pack_partials(lis, col0):
                for i, li in enumerate(lis):
                    cols = lcols[li]
                    assert cols == list(range(cols[0], cols[-1] + 1))
                    nc.vector.tensor_reduce(pm[:, col0 + i:col0 + i + 1],
                                            partials[:, cols[0]:cols[-1] + 1],
                                            mybir.AxisListType.X, OP.add)

            def _scale_par(col0, ncols):
                sl = slice(col0, col0 + ncols)
                nc.gpsimd.partition_all_reduce(pmsr[:, sl], pms[:, sl],
                                               channels=P,
                                               reduce_op=bass_isa.ReduceOp.add)

            def _scale_dve(col0, ncols):
                sl = slice(col0, col0 + ncols)
                nc.vector.tensor_tensor(mwb4[:, sl], pmsr[:, sl],
                                        invn[:, sl], OP.mult)
                nc.vector.tensor_scalar(mwb4[:, sl], mwb4[:, sl], float(EPS),
                                        None, OP.max)
                nc.vector.reciprocal(swb4[:, sl], mwb4[:, sl])

            def _scale_post(col0, ncols):
                _scale_par(col0, ncols)
                _scale_dve(col0, ncols)

            def _quant_issue(lis, dma):
                out = []
                for ri, (li, g, c0, w) in enumerate(runs4):
                    if li not in lis:
                        continue
                    off = wsh_off(li, g) + c0 * P
                    wrun = wqpool.tile([P, RUN // 2 * OBW], F32, tag="wq",
                                       name=f"wrC{ri}")
                    dma(wrun[:, :w], wsh[off:off + P * w].rearrange(
                        "(p f) -> p f", p=P))
                    out.append((ri, wrun, w))
                return out

            def _quant_compute(issued, dve_only=False):
                for ri, wrun, w in issued:
                    li, g, c0, _ = runs4[ri]
                    if dve_only:
                        nc.vector.tensor_scalar(wrun[:, :w], wrun[:, :w],
                                                swb[li], MAGIC,
                                                OP.mult, OP.add)
                    else:
                        nc.scalar.activation(wrun[:, :w], wrun[:, :w],
                                             AF.Copy, bias=MAGIC,
                                             scale=swb[li])
                    nc.vector.tensor_scalar(wrun[:, :w], wrun[:, :w], MAGIC,
                                            1.0, OP.subtract, OP.min)
                    qblk = qopool.tile([P, RUN // 2 * OBW], FP8, tag="qo",
                                       name=f"qo{ri}")
                    nc.vector.tensor_scalar(qblk[:, :w], wrun[:, :w], -1.0,
                                            None, OP.max)
                    soff = c0 * P
                    nc.gpsimd.dma_start(
                        stage[(li, g)][soff:soff + P * w].rearrange(
                            "(p f) -> p f", p=P),
                        qblk[:, :w])
                    if c0 + w == layers[li]["gw"]:
                        # bf16-typed APs double the per-descriptor payload of
                        # the AllGather (byte-identical data)
                        nc.gpsimd.collective_compute(
                            "AllGather", OP.bypass,
                            ins=[stage[(li, g)][:].bitcast(BF16)],
                            outs=[image[(li, g)][:].bitcast(BF16)],
                            replica_groups=[list(range(N_CORES))])

            # S1: prewarm gpsimd DSP lib; L0/L1 abs reads + x reads in flight
            prew = small.tile([P, 4], F32, tag="prew", name="prew")
            nc.any.memset(prew[:], 0)
            nc.gpsimd.partition_all_reduce(prew[:], prew[:], channels=P,
                                           reduce_op=bass_isa.ReduceOp.add)
            issued0 = _abs_issue([0], nc.sync.dma_start)
            xts = []
            for t in range(T):
                xt = ypool.tile([P, D0], F32, tag="y", name=f"xt{t}")
                nc.sync.dma_start(xt[:], xs[t * P:(t + 1) * P, :])
                xts.append(xt)
            issued1 = _abs_issue([1], nc.scalar.dma_start)

            # S2: L0 sums -> AR0 launched immediately (rides the barrier)
            _abs_compute(issued0)
            _pack_partials([0], 0)
            nc.gpsimd.dma_start(ar_in0[:], pm[:, 0:1])
            nc.gpsimd.collective_compute(
                "AllReduce", OP.add,
                ins=[ar_in0.opt()], outs=[ar_out0.opt()],
                replica_groups=[list(range(N_CORES))])
            # S3: L1 sums
            _abs_compute(issued1)

            # S3.5: input activation quant (x already in flight)
            xqT_cur = []
            am0s = []
            for t in range(T):
                xt = xts[t]
                am = small.tile([P, 1], F32, tag=f"am0_{t}", name=f"am0_{t}")
                nc.vector.tensor_reduce(am[:], xt[:], mybir.AxisListType.X,
                                        OP.max, apply_absolute_value=True)
                nc.vector.tensor_scalar(am[:], am[:], float(EPS), None, OP.max)
                sc = small.tile([P, 1], F32, tag=f"s0_{t}", name=f"s0_{t}")
                nc.vector.tensor_scalar(sc[:], am[:], 1.0 / 127.0, None,
                                        OP.mult)
                nc.vector.reciprocal(sc[:], sc[:])
                xq0 = xqnpool.tile([P, D0], BF16, tag="xqn", name=f"xq0_{t}")
                for ch in range(D0 // OBW):
                    uu = upool.tile([P, OBW], F32, tag="u", name=f"u0_{t}_{ch}")
                    nc.scalar.activation(uu[:], xt[:, ch * OBW:(ch + 1) * OBW],
                                         AF.Copy, bias=MAGIC, scale=sc[:])
                    nc.vector.tensor_scalar(xq0[:, ch * OBW:(ch + 1) * OBW],
                                            uu[:], MAGIC, None, OP.subtract)
                xqT0 = xqT0pool.tile([P, n_ic0, P], BF16, tag="xqT0",
                                     name=f"xqT0_{t}")
                nc.scalar.dma_start_transpose(xqT0[:], xq0[:])
                xqT_cur.append(xqT0)
                am0s.append(am)

            # S4: AR0 readback + L0 scale
            nc.sync.dma_start(pms[:, 0:1], ar_out0[:])
            _scale_post(0, 1)
            # S5: L2/L3 abs reads (after AR0 on the gpsimd queue)
            issued23 = _abs_issue([2, 3], nc.gpsimd.dma_start)
            # S7: quant L0 + its AllGather
            q0 = _quant_issue([0], nc.sync.dma_start)
            _quant_compute(q0)
            # S8: L1 quant reads (data-independent, fire early)
            q1 = _quant_issue([1], nc.sync.dma_start)
            c_cur = []
            for t in range(T):
                c0t = small.tile([P, 1], F32, tag=f"c0_{t}", name=f"c0_{t}")
                nc.vector.scalar_tensor_tensor(c0t[:], am0s[t][:], 1.0 / 127.0,
                                               mwb[0], OP.mult, OP.mult)
                c_cur.append(c0t)
            # S9: L2/L3 sums (DVE-only: the scalar queue must stay clear
            # for quant-L0 -> AG-L0 and then L0's evictions)
            _abs_compute(issued23, dve_only=True)
            # S10: AR123 + L1 quant (pure DVE so L0 evictions aren't blocked
            # behind it on the scalar queue) + L1 AllGather chunks
            _pack_partials([1, 2, 3], 1)
            nc.gpsimd.dma_start(ar_in123[:], pm[:, 1:4])
            nc.gpsimd.collective_compute(
                "AllReduce", OP.add,
                ins=[ar_in123.opt()], outs=[ar_out123.opt()],
                replica_groups=[list(range(N_CORES))])
            nc.gpsimd.dma_start(pms[:, 1:4], ar_out123[:])
            _scale_par(1, 3)
            _scale_dve(1, 3)
            _quant_compute(q1[:1], dve_only=True)
            q23 = [None]
            q23[0] = _quant_issue([2, 3], nc.gpsimd.dma_start)
            _quant_compute(q1[1:], dve_only=True)

            # ---------------- Stage D: main pass ----------------
            for L in layers:
                if T != 4:
                    phases = [tuple(range(T))]
                elif L["li"] in (1, 2):
                    # L1/L2 AllGather chunks trail the matmuls; a 3-tile first
                    # phase consumes each chunk slower than the ring delivers
                    phases = [(0, 1, 2), (3,)]
                else:
                    phases = [(0, 1), (2, 3)]
                li, n_ic, n_ob, q = L["li"], L["n_ic"], L["n_ob"], L["q"]
                opg, gw = L["opg"], L["gw"]
                dout, dreal = L["dout"], L["dreal"]
                is_last = (li == 3)
                S = P * gw  # per-rank image block size (elements)

                ys = {}
                bns = {}
                xqT_next = {}
                c_next = {}

                for ph, ptiles in enumerate(phases):
                    for ob in range(n_ob):
                        g, obl = divmod(ob, opg)
                        ow = L["ob_w"][ob]
                        wp = wpool.tile([P, n_ic, OBW], FP8, tag="wp",
                                        name=f"wp{li}_{ph}_{ob}")
                        img = image[(li, g)]
                        for rb in range(N_CORES):
                            src = img[rb * S:(rb + 1) * S].rearrange(
                                "(p f) -> p f", p=P)
                            nc.sync.dma_start(
                                wp[:, rb * q:(rb + 1) * q, :],
                                src[:, obl * q * OBW:(obl + 1) * q * OBW])
                        for t in ptiles:
                            if ob == 0:
                                ys[t] = ypool.tile([P, dreal], F32, tag="y",
                                                   name=f"y{li}_{t}")
                                if not is_last:
                                    bns[t] = small.tile(
                                        [P, n_ob * 6], F32, tag=f"bn{t}",
                                        name=f"bn{li}_{t}")
                            ps = psum.tile([P, OBW], F32, tag="ps",
                                           name=f"ps{li}_{ob}_{t}")
                            for c in range(n_ic):
                                nc.tensor.matmul(
                                    ps[:], xqT_cur[t][:, c, :], wp[:, c, :],
                                    start=(c == 0), stop=(c == n_ic - 1))
                            dst = ys[t][:, ob * OBW:ob * OBW + ow]
                            if not is_last:
                                nc.scalar.activation(dst, ps[:, :ow], AF.Copy,
                                                     bias=0.0,
                                                     scale=c_cur[t][:])
                                nc.vector.bn_stats(
                                    bns[t][:, ob * 6:(ob + 1) * 6], dst)
                            else:
                                nc.scalar.activation(dst, ps[:, :ow],
                                                     AF.Sigmoid, bias=0.0,
                                                     scale=c_cur[t][:])

                    if li == 1 and ph == 0:
                        # L2/L3 weight quant + AllGathers: before the phase
                        # tails so AG-L2a triggers as early as possible
                        _quant_compute(q23[0])
                    # ---- phase tail ----
                    for t in ptiles:
                        if is_last:
                            nc.vector.tensor_scalar(ys[t][:, 0:OBINS],
                                                    ys[t][:, 0:OBINS],
                                                    float(OBINS - 1), 1.0,
                                                    OP.mult, OP.add)
                            nc.scalar.dma_start(mz_out[t * P:(t + 1) * P, :],
                                                ys[t][:, 0:OBINS])
                            nc.vector.tensor_scalar(ys[t][:, OBINS:2 * OBINS],
                                                    ys[t][:, OBINS:2 * OBINS],
                                                    100.0, None, OP.mult)
                            nc.scalar.dma_start(ii_out[t * P:(t + 1) * P, :],
                                                ys[t][:, OBINS:2 * OBINS])
                            continue
                        # LN + SiLU + act quant + transpose
                        n_ic_next = layers[li + 1]["n_ic"]
                        mv = small.tile([P, 2], F32, tag=f"mv{t}",
                                        name=f"mv{li}_{t}")
                        nc.vector.bn_aggr(mv[:], bns[t][:])
                        v = small.tile([P, 1], F32, tag=f"vvar{t}",
                                       name=f"v{li}_{t}")
                        nc.vector.tensor_scalar(v[:], mv[:, 1:2], float(EPS),
                                                None, OP.add)
                        istd = _rsqrt_newton(nc, small, v[:])
                        nmi = small.tile([P, 1], F32, tag=f"nmi{t}",
                                         name=f"nmi{li}_{t}")
                        nc.vector.scalar_tensor_tensor(nmi[:], mv[:, 0:1],
                                                       -1.0, istd[:],
                                                       OP.mult, OP.mult)
                        nc.scalar.activation(ys[t][:], ys[t][:], AF.Identity,
                                             bias=nmi[:], scale=istd[:])
                        amsl = small.tile([P, 8], F32, tag=f"amsl{t}",
                                          name=f"amsl{li}_{t}")
                        n_chk = dout // OBW
                        for ch in range(n_chk):
                            sl = ys[t][:, ch * OBW:(ch + 1) * OBW]
                            sg = sgpool.tile([P, OBW], F32, tag="sg",
                                             name=f"sg{li}_{t}_{ch}")
                            nc.scalar.activation(sg[:], sl, AF.Tanh,
                                                 bias=0.0, scale=0.5)
                            nc.vector.tensor_scalar(sg[:], sg[:], 0.5, 0.5,
                                                    OP.mult, OP.add)
                            nc.vector.tensor_tensor(sl, sl, sg[:], OP.mult)
                            nc.vector.tensor_reduce(amsl[:, ch:ch + 1], sl,
                                                    mybir.AxisListType.X,
                                                    OP.max,
                                                    apply_absolute_value=True)
                        am = small.tile([P, 1], F32, tag=f"amn{t}",
                                        name=f"am{li}_{t}")
                        nc.vector.tensor_reduce(am[:], amsl[:, :n_chk],
                                                mybir.AxisListType.X, OP.max)
                        nc.vector.tensor_scalar(am[:], am[:], float(EPS),
                                                None, OP.max)
                        sc = small.tile([P, 1], F32, tag=f"scn{t}",
                                        name=f"sc{li}_{t}")
                        nc.vector.tensor_scalar(sc[:], am[:], 1.0 / 127.0,
                                                None, OP.mult)
                        nc.vector.reciprocal(sc[:], sc[:])
                        cn = small.tile([P, 1], F32, tag=f"cn{t}",
                                        name=f"c{li + 1}_{t}")
                        nc.vector.scalar_tensor_tensor(cn[:], am[:],
                                                       1.0 / 127.0,
                                                       mwb[li + 1],
                                                       OP.mult, OP.mult)
                        c_next[t] = cn
                        xqn = xqnpool.tile([P, dout], BF16, tag="xqn",
                                           name=f"xqn{li}_{t}")
                        for ch in range(n_chk):
                            uu = upool.tile([P, OBW], F32, tag="u",
                                            name=f"ur{li}_{t}_{ch}")
                            nc.scalar.activation(
                                uu[:], ys[t][:, ch * OBW:(ch + 1) * OBW],
                                AF.Copy, bias=MAGIC, scale=sc[:])
                            nc.vector.tensor_scalar(
                                xqn[:, ch * OBW:(ch + 1) * OBW],
                                uu[:], MAGIC, None, OP.subtract)
                        xT = xqTpool.tile([P, n_ic_next, P], BF16, tag="xqT",
                                          name=f"xT{li}_{t}")
                        nc.scalar.dma_start_transpose(xT[:], xqn[:])
                        xqT_next[t] = xT

                if not is_last:
                    xqT_cur = [xqT_next[t] for t in range(T)]
                    c_cur = [c_next[t] for t in range(T)]

    nc.compile()
    return nc


def prepare_inputs(cfg, x, W0, W1, W2, W3):
    """Host-side sharding (pure layout, no arithmetic): per-core input maps.

    wsh layout per rank r: for layer li, for ob-group g, a [128, gw] fp32
    block (partition-major flat) whose cols are [ob-local][j][o] with
    j in [0, q): value = W_li^T[(r*q+j)*128 + p, ob*512 + o].
    """
    layers, per_rank, T = _plan(cfg)
    b_core = T * P
    Ws = [np.asarray(W0), np.asarray(W1), np.asarray(W2), np.asarray(W3)]
    WTs = []
    for L, W in zip(layers, Ws):
        WT = np.zeros((L["din"], L["dout"]), dtype=np.float32)
        WT[:, :L["dreal"]] = W.T
        WTs.append(WT)

    shards = [np.empty(per_rank, dtype=np.float32) for _ in range(N_CORES)]
    for L in layers:
        li, q, opg, gw = L["li"], L["q"], L["opg"], L["gw"]
        WT = WTs[li]
        for r in range(N_CORES):
            for g in range(L["n_g"]):
                blk = np.empty((P, gw), dtype=np.float32)
                for obl in range(opg):
                    ob = g * opg + obl
                    for j in range(q):
                        ic = r * q + j
                        blk[:, (obl * q + j) * OBW:(obl * q + j + 1) * OBW] = \
                            WT[ic * P:(ic + 1) * P, ob * OBW:(ob + 1) * OBW]
                off = L["woff"] + g * P * gw
                shards[r][off:off + P * gw] = blk.reshape(-1)
    x = np.asarray(x, dtype=np.float32)
    in_maps = []
    for r in range(N_CORES):
        in_maps.append(dict(
            xs=np.ascontiguousarray(x[r * b_core:(r + 1) * b_core]),
            wsh=shards[r],
        ))
    return in_maps


_NC_CACHE = {}


def _get_nc(cfg_key):
    if cfg_key not in _NC_CACHE:
        _NC_CACHE[cfg_key] = build(dict(cfg_key))
    return _NC_CACHE[cfg_key]


def run(cfg, x, W0, W1, W2, W3, trace=False):
    layers, per_rank, T = _plan(cfg)
    b_core = T * P
    nc = _get_nc(tuple(sorted(cfg.items())))
    in_maps = prepare_inputs(cfg, x, W0, W1, W2, W3)
    res = run_bass_kernel_spmd(nc, in_maps, core_ids=list(range(N_CORES)),
                               trace=trace)
    mz = np.concatenate([res.results[r]["mz"] for r in range(N_CORES)], axis=0)
    ii = np.concatenate([res.results[r]["ii"] for r in range(N_CORES)], axis=0)
    return (mz, ii), res


def kernel(x, W0, W1, W2, W3, g0, b0, g1, b1, g2, b2):
    """Full-input entry point. g/b are identity (ones/zeros) in this problem's
    setup; LayerNorm affine is a no-op and is validated here."""
    for g in (g0, g1, g2):
        assert np.allclose(np.asarray(g), 1.0), "non-identity LN gain unsupported"
    for b in (b0, b1, b2):
        assert np.allclose(np.asarray(b), 0.0), "non-zero LN bias unsupported"
    (mz, ii), _ = run(FULL_CFG, x, W0, W1, W2, W3, trace=False)
    return (mz, ii)
